# revision 42
# baseline (speedup 1.0000x reference)
"""Trainium2 Bass kernel for 16-head MultiHeadAttention.

Problem shapes (hardcoded): B=2, L=2048, D=1024, H=16, DK=64, fp32 I/O.

Sharding over 8 cores: core c handles batch b=c//4 and head-group g=c%4
(4 heads, 256 of the 1024 QKV columns).  All matmul operands are bf16
(f32 PSUM accumulation); X is supplied PRE-TRANSPOSED (X[b].T, bf16) by
the host, so no on-chip transposes are needed.

Per core:
  load x^T [128,8dc,2048] per l-quarter (sync queue), weights (scalar
  queue), biases (SWDGE ring)
  KT/QT = W^T X^T + b  (DVE tensor_scalar_add evicts PSUM, casts bf16;
  ACT is reserved exclusively for the softmax exp)
  Vaug  = X Wvaug + bvaug ([1 | V] per head: ones col FIRST so the
  softmax denominator lands on PSUM partition 0)     [128,16lt,260]
  attention per 512-col q-subblock per head-PAIR (2m, 2m+1): the two
  heads' S^T matmuls use disjoint 64-row PE quadrants (tile_position
  (0,0) vs (64,0)) emitted back-to-back (they execute CONCURRENTLY on
  the PE array), sharing one [128,2,512] PSUM tile (double-buffered so
  exp(kti) overlaps S(kti+1)); ONE exp (ACT, scale 1/8) covers both
  heads; AV accumulates per-head [65,512] (row 0 = denominator)
  normalize: recip(denominator row) -> gpsimd partition_broadcast ->
  multiply straight out of PSUM (evacuates otp)
  exchange: one AllToAll per (half, pair) = 4 chunks of 256KB, so the
  collective pipeline overlaps attention compute and only a small
  chunk remains on the critical-path tail
  Y[128,1024] = oall^T Wo + bo per (half, batch), with the contraction
  ordered pair-0-chunks-first so Y can start before the pair-1 chunk
  lands; core c outputs rows {128c, 1024+128c}.

Q^T for half 1 and Y for half 0 are emitted inside the attention
stream (PE has slack under the ACT exp), reusing the attention PSUM
slot via the shared "sp" tag.
"""

import numpy as np
import ml_dtypes
from contextlib import ExitStack

import concourse.bass as bass
import concourse.bacc as bacc
import concourse.mybir as mybir
import concourse.tile as tile
from concourse.bass_utils import run_bass_kernel_spmd

F32 = mybir.dt.float32
BF16 = mybir.dt.bfloat16
AF = mybir.ActivationFunctionType

B, L, D, H, DK = 2, 2048, 1024, 16, 64
NCORES = 8
NH = 4              # heads per core
CPC = NH * DK       # 256 qkv cols per core
VA = NH * (DK + 1)  # 260, V-aug width (per-head [1 | V])
LT = L // 128       # 16 k-chunks
DCH = D // 128      # 8 d-chunks
NQ = 4              # l-quarters
QW = L // NQ        # 512


def _emit(tc, nc, x, wq, bq, wk, wk1, bk, wv, bv, wo, bo, out):
    with ExitStack() as es:
        # ---------------- persistent pools ----------------
        const = es.enter_context(tc.tile_pool(name="const", bufs=1))
        wq_sb = const.tile([128, DCH, CPC], BF16)
        wk0_sb = const.tile([128, DCH, 128], BF16)
        wk1_sb = const.tile([128, DCH, 128], BF16)
        wv_sb = const.tile([128, DCH, VA], BF16)
        wo_sb = const.tile([128, DCH, D], BF16)
        bq_sb = const.tile([128, 2, 1], F32)
        bk_sb = const.tile([128, 2, 1], F32)
        bv_bc = const.tile([128, VA], F32)
        bo_bc = const.tile([128, D], F32)

        proj = es.enter_context(tc.tile_pool(name="proj", bufs=1))
        # X^T quarter-major: [p, lq, dc, 512] so each quarter loads as
        # one contiguous-8KB-per-partition DMA
        xt = proj.tile([128, NQ, DCH, QW], BF16)
        qt = proj.tile([128, 2, L], BF16)       # Q^T, c-chunk m rows
        kt = proj.tile([128, 2, L], BF16)       # K^T
        vaug = proj.tile([128, LT, VA], BF16)   # [1 | V] per k-chunk

        # ---------------- input DMAs ----------------
        # weights (host-pretiled [128, DCH, cols] so lines are 4-16KB)
        # + odd x quarters on the scalar queue (K first: first PE
        # consumer)
        xr = x.ap()
        # wk split by m-half (two contiguous host params) across two
        # queues so K proj starts early
        nc.scalar.dma_start(wk0_sb, wk.ap())
        nc.gpsimd.dma_start(wk1_sb, wk1.ap())
        nc.scalar.dma_start(wq_sb, wq.ap())
        nc.scalar.dma_start(wv_sb, wv.ap())
        nc.scalar.dma_start(wo_sb, wo.ap())
        # biases on the SWDGE ring (tiny, out of the way)
        nc.gpsimd.dma_start(
            bk_sb, bk.ap().rearrange("(m p) o -> p m o", p=128))
        nc.gpsimd.dma_start(
            bq_sb, bq.ap().rearrange("(m p) o -> p m o", p=128))
        bv_ap = bv.ap()
        nc.gpsimd.dma_start(
            bv_bc, bass.AP(tensor=bv_ap.tensor, offset=bv_ap.offset,
                           ap=[[0, 128]] + list(bv_ap.ap[1:])))
        bo_ap = bo.ap()
        nc.gpsimd.dma_start(
            bo_bc, bass.AP(tensor=bo_ap.tensor, offset=bo_ap.offset,
                           ap=[[0, 128]] + list(bo_ap.ap[1:])))
        # x^T quarters on the sync queue in consumption order; host
        # supplies x pre-tiled [NQ, 128, DCH, QW] so lines are
        # 8KB/partition
        for lq in range(NQ):
            nc.sync.dma_start(xt[:, lq], xr[lq])

        # ---------------- phase P: K, Q(half0), V ----------------
        with ExitStack() as ph:
            psA = ph.enter_context(tc.tile_pool(name="psA", bufs=3,
                                                space="PSUM"))
            psV = ph.enter_context(tc.tile_pool(name="psV", bufs=3,
                                                space="PSUM"))

            def proj_qk(w_of, b_sb, dst, lq):
                for m in range(2):
                    pq = psA.tile([128, QW], F32, tag="pq",
                                  name=f"pq{lq}{m}")
                    for dc in range(DCH):
                        nc.tensor.matmul(
                            pq, w_of(m, dc), xt[:, lq, dc, :],
                            start=(dc == 0), stop=(dc == DCH - 1))
                    nc.vector.tensor_scalar_add(
                        dst[:, m, lq * QW:(lq + 1) * QW], pq, b_sb[:, m, :])

            def wk_of(m, dc):
                return (wk0_sb if m == 0 else wk1_sb)[:, dc, :]

            def wq_of(m, dc):
                return wq_sb[:, dc, m * 128:(m + 1) * 128]

            for lq in range(NQ):
                proj_qk(wk_of, bk_sb, kt, lq)
            for lq in range(2):
                proj_qk(wq_of, bq_sb, qt, lq)
            for lt in range(LT):
                pv = psV.tile([128, VA], F32, tag="pv", name=f"pv{lt}")
                for dc in range(DCH):
                    nc.tensor.matmul(
                        pv, xt[:, lt // 4, dc,
                               (lt % 4) * 128:(lt % 4 + 1) * 128],
                        wv_sb[:, dc, :],
                        start=(dc == 0), stop=(dc == DCH - 1))
                nc.vector.tensor_add(vaug[:, lt, :], pv, bv_bc)

        # ---------------- attention + exchange + Y ----------------
        dramp = es.enter_context(tc.tile_pool(name="dramp", bufs=1,
                                              space="DRAM"))
        # tiny dummy AllToAll issued while projections run: it absorbs
        # the framework's init barrier + inter-core launch skew so the
        # first REAL exchange starts with a warmed-up cc stream
        # (collectives can't read IO tensors, so stage 512B via DMA)
        dummi = dramp.tile([NCORES, 16], F32, name="dummi")
        dummo = dramp.tile([NCORES, 16], F32, name="dummo")
        nc.gpsimd.dma_start(
            dummi, bo.ap()[:, 0:128].rearrange("o (s c) -> s (o c)",
                                               s=NCORES))
        nc.gpsimd.collective_compute(
            "AllToAll", mybir.AluOpType.bypass,
            replica_groups=[list(range(NCORES))],
            ins=[dummi.opt()], outs=[dummo.opt()])
        # separate DRAM tiles per (half, pair) so the dependency
        # tracker never serializes pair p+1's packs behind pair p's
        # AllToAll: [dest core][head-col(2x64)][q-row 128]
        ain = {(hf, p): dramp.tile([NCORES, 128, 128], BF16,
                                   name=f"ain{hf}{p}")
               for hf in range(2) for p in range(2)}
        aout = {(hf, p): dramp.tile([NCORES, 128, 128], BF16,
                                    name=f"aout{hf}{p}")
                for hf in range(2) for p in range(2)}

        psS = es.enter_context(tc.tile_pool(name="psS", bufs=2,
                                            space="PSUM"))
        psO = es.enter_context(tc.tile_pool(name="psO", bufs=1,
                                            space="PSUM"))
        psX = es.enter_context(tc.tile_pool(name="psX", bufs=1,
                                            space="PSUM"))
        upool = es.enter_context(tc.tile_pool(name="upool", bufs=3))
        npool = es.enter_context(tc.tile_pool(name="npool", bufs=2))
        opool = es.enter_context(tc.tile_pool(name="opool", bufs=2))
        ypool = es.enter_context(tc.tile_pool(name="ypool", bufs=2))
        outap = out.ap()

        def q_half1_filler():
            # Q^T proj for half 1, via the 2-bank aux PSUM slot;
            # yielded one matmul at a time so it interleaves into the
            # attention stream's PE slack
            for lq in (2, 3):
                pq = psX.tile([128, 2, QW], F32, tag="aux",
                              name=f"pqh1{lq}")
                for m in range(2):
                    for dc in range(DCH):
                        nc.tensor.matmul(
                            pq[:, m, :], wq_sb[:, dc, m * 128:(m + 1) * 128],
                            xt[:, lq, dc, :],
                            start=(dc == 0), stop=(dc == DCH - 1))
                        yield
                for m in range(2):
                    nc.vector.tensor_scalar_add(
                        qt[:, m, lq * QW:(lq + 1) * QW], pq[:, m, :],
                        bq_sb[:, m, :])
                yield

        def y_filler(half):
            # output projection for one half, yielded in matmul-sized
            # units; pair-0 chunks first so Y can start before the
            # pair-1 AllToAll lands
            for b in range(B):
                oall = opool.tile([128, 2, 4, 128], BF16, tag="oall",
                                  name=f"oall{half}{b}")
                for pr in range(2):
                    nc.scalar.dma_start(
                        oall[:, pr],
                        aout[half, pr][4 * b:4 * b + 4, :, :].rearrange(
                            "s p l -> p s l"))
                yield
                yp = psX.tile([128, 2, 512], F32, tag="aux",
                              name=f"yp{half}{b}")
                for pr in range(2):
                    for s4 in range(4):
                        j = s4 * 2 + pr
                        for nn in range(2):
                            nc.tensor.matmul(
                                yp[:, nn, :], oall[:, pr, s4, :],
                                wo_sb[:, j, nn * 512:(nn + 1) * 512],
                                start=(pr == 0 and s4 == 0),
                                stop=(pr == 1 and s4 == 3))
                        yield
                y_sb = ypool.tile([128, D], F32, tag="ysb",
                                  name=f"ysb{half}{b}")
                for nn in range(2):
                    nc.vector.tensor_add(
                        y_sb[:, nn * 512:(nn + 1) * 512], yp[:, nn, :],
                        bo_bc[:, nn * 512:(nn + 1) * 512])
                nc.scalar.dma_start(outap[b, half, :, :], y_sb)
                yield

        def drain(filler):
            if filler is not None:
                for _ in filler:
                    pass

        def attn_block(half, p, qq, filler=None):
            # 512 q-cols: global rows [half*1024 + qq*512, +512) ->
            # dest cores 4qq..4qq+3 of ain[half][p]
            q0 = half * 1024 + qq * QW
            he, ho = 2 * p, 2 * p + 1
            otp_e = psO.tile([65, QW], F32, tag="otpe",
                             name=f"oe{half}{p}{qq}")
            otp_o = psO.tile([65, QW], F32, tag="otpo",
                             name=f"oo{half}{p}{qq}")
            def emit_s(kti):
                # flat [128,1024] (cols 0-511 head-e, 512-1023 head-o):
                # 1-D APs keep the ACT exp at full rate.  S^T for both
                # heads: disjoint 64-row PE quadrants, back-to-back
                # emission (they execute concurrently)
                sp = psS.tile([128, 2 * QW], F32, tag="sp",
                              name=f"sp{half}{p}{qq}{kti}")
                for xx, r0 in ((0, 0), (1, 64)):
                    nc.tensor.matmul(
                        sp[:, xx * QW:(xx + 1) * QW],
                        kt[r0:r0 + 64, p, kti * 128:(kti + 1) * 128],
                        qt[r0:r0 + 64, p, q0:q0 + QW],
                        start=True, stop=True)
                return sp

            # software pipeline: S(kti+1) is emitted BEFORE AV(kti) so
            # the PE never sits in the exp(kti)->AV(kti) latency chain
            # and exp(kti+1)'s input is ready the moment exp(kti) ends
            sp = emit_s(0)
            for kti in range(LT):
                u = upool.tile([128, 2 * QW], BF16, tag="u",
                               name=f"u{half}{p}{qq}{kti}")
                nc.scalar.activation(u, sp, AF.Exp, scale=0.125)
                if kti + 1 < LT:
                    sp = emit_s(kti + 1)
                for xx, otp, h in ((0, otp_e, he), (1, otp_o, ho)):
                    nc.tensor.matmul(
                        otp, vaug[:, kti, h * 65:(h + 1) * 65],
                        u[:, xx * QW:(xx + 1) * QW],
                        start=(kti == 0), stop=(kti == LT - 1))
                if filler is not None:
                    next(filler, None)
            # normalization: row 0 of otp is the denominator
            for xx, otp in ((0, otp_e), (1, otp_o)):
                rec1 = npool.tile([1, QW], F32, tag="rec",
                                  name=f"r{half}{p}{qq}{xx}")
                nc.vector.reciprocal_approx_fast(rec1, otp[0:1, :])
                rbc = npool.tile([65, QW], F32, tag="rbc",
                                 name=f"rb{half}{p}{qq}{xx}")
                nc.gpsimd.partition_broadcast(rbc, rec1, channels=65)
                otn = npool.tile([65, QW], BF16, tag="otn",
                                 name=f"on{half}{p}{qq}{xx}")
                nc.vector.tensor_mul(otn, otp, rbc)
                dst = ain[half, p][4 * qq:4 * qq + 4,
                                   xx * 64:(xx + 1) * 64, :].rearrange(
                    "s p l -> p s l")
                nc.sync.dma_start(
                    dst, otn[1:65, :].rearrange("p (s l) -> p s l", s=4))

        def attn_pair(half, p, filler=None):
            attn_block(half, p, 0, filler)
            attn_block(half, p, 1, filler)
            drain(filler)
            nc.gpsimd.collective_compute(
                "AllToAll", mybir.AluOpType.bypass,
                replica_groups=[list(range(NCORES))],
                ins=[ain[half, p].opt()], outs=[aout[half, p].opt()])

        attn_pair(0, 0)
        attn_pair(0, 1, filler=q_half1_filler())
        attn_pair(1, 0)
        attn_pair(1, 1, filler=y_filler(0))
        drain(y_filler(1))


_CACHED_NC = None


def _build_program():
    global _CACHED_NC
    if _CACHED_NC is not None:
        return _CACHED_NC
    nc = bacc.Bacc(None, target_bir_lowering=False, debug=False,
                   num_devices=NCORES)
    x = nc.declare_dram_parameter("x", [NQ, 128, DCH, QW], BF16,
                                  isOutput=False)
    wq = nc.declare_dram_parameter("wq", [128, DCH, CPC], BF16,
                                   isOutput=False)
    bq = nc.declare_dram_parameter("bq", [CPC, 1], F32, isOutput=False)
    wk = nc.declare_dram_parameter("wk", [128, DCH, 128], BF16,
                                   isOutput=False)
    wk1 = nc.declare_dram_parameter("wk1", [128, DCH, 128], BF16,
                                    isOutput=False)
    bk = nc.declare_dram_parameter("bk", [CPC, 1], F32, isOutput=False)
    wv = nc.declare_dram_parameter("wv", [128, DCH, VA], BF16,
                                   isOutput=False)
    bv = nc.declare_dram_parameter("bv", [1, VA], F32, isOutput=False)
    wo = nc.declare_dram_parameter("wo", [128, DCH, D], BF16,
                                   isOutput=False)
    bo = nc.declare_dram_parameter("bo", [1, D], F32, isOutput=False)
    out = nc.declare_dram_parameter("out", [B, 2, 128, D], F32,
                                    isOutput=True)

    with tile.TileContext(nc) as tc:
        _emit(tc, nc, x, wq, bq, wk, wk1, bk, wv, bv, wo, bo, out)
    nc.finalize()
    _CACHED_NC = nc
    return nc


def _make_in_maps(X, WQ, bQ, WK, bK, WV, bV, WO, bO):
    bf = ml_dtypes.bfloat16

    def tile_w(w):  # [D, cols] -> [128, DCH, cols] with row dc*128+p
        return np.ascontiguousarray(
            w.reshape(DCH, 128, -1).transpose(1, 0, 2).astype(bf))

    X = np.asarray(X, np.float32)
    WQ = np.asarray(WQ, np.float32)
    WK = np.asarray(WK, np.float32)
    WV = np.asarray(WV, np.float32)
    WO = tile_w(np.asarray(WO, np.float32))
    bO = np.asarray(bO, np.float32).reshape(1, D)
    # x^T tiled [lq, p, dc, c] = X[b][lq*512+c, dc*128+p]
    xts = [np.ascontiguousarray(
        X[b].T.reshape(DCH, 128, NQ, QW).transpose(2, 1, 0, 3).astype(bf))
        for b in range(B)]
    in_maps = []
    for c in range(NCORES):
        b, g = c // 4, c % 4
        cs = slice(CPC * g, CPC * (g + 1))
        wva = np.zeros((D, VA), np.float32)
        bva = np.zeros((1, VA), np.float32)
        for h in range(NH):
            # ones column FIRST, then the 64 V columns
            bva[0, h * 65] = 1.0
            wva[:, h * 65 + 1:h * 65 + 65] = WV[:, CPC * g + 64 * h:
                                                CPC * g + 64 * (h + 1)]
            bva[0, h * 65 + 1:h * 65 + 65] = bV[CPC * g + 64 * h:
                                                CPC * g + 64 * (h + 1)]
        in_maps.append({
            "x": xts[b],
            "wq": tile_w(WQ[:, cs]),
            "bq": np.ascontiguousarray(np.asarray(bQ, np.float32)[cs]
                                       .reshape(CPC, 1)),
            "wk": tile_w(WK[:, cs.start:cs.start + 128]),
            "wk1": tile_w(WK[:, cs.start + 128:cs.stop]),
            "bk": np.ascontiguousarray(np.asarray(bK, np.float32)[cs]
                                       .reshape(CPC, 1)),
            "wv": tile_w(wva),
            "bv": bva,
            "wo": WO,
            "bo": np.ascontiguousarray(bO),
        })
    return in_maps


def _assemble(results):
    full = np.empty((B, L, D), np.float32)
    for c in range(NCORES):
        o = results[c]["out"]  # [B, 2, 128, D]
        for b in range(B):
            full[b, 128 * c:128 * (c + 1), :] = o[b, 0]
            full[b, 1024 + 128 * c:1024 + 128 * (c + 1), :] = o[b, 1]
    return full


def run(inputs, trace=False):
    nc = _build_program()
    in_maps = _make_in_maps(**inputs)
    res = run_bass_kernel_spmd(nc, in_maps, list(range(NCORES)), trace=trace)
    return _assemble(res.results), res


def kernel(X, WQ, bQ, WK, bK, WV, bV, WO, bO):
    out, _ = run(dict(X=X, WQ=WQ, bQ=bQ, WK=WK, bK=bK, WV=WV, bV=bV,
                      WO=WO, bO=bO))
    return out
